# revision 1
# baseline (speedup 1.0000x reference)
"""Causal multi-head attention (B=4, S=2048, D=1024, H=16, hd=64) on 8 TRN2
NeuronCores.

Sharding: core c = (batch b = c//2, head-group g = c%2). Each core computes
QKV projections for its 8 heads (Megatron column-split), causal attention,
and a partial out-projection (row-split); the host sums the two head-group
partials per batch and adds the bias.

On-device layout (bf16 compute, fp32 PSUM accumulation):
  xT  [p, q-block, din-subtile, 512]  x[b]^T pre-tiled on host so each
        input DMA moves 8KB-contiguous runs per partition (descriptor-
        efficient); same for wq/wk (head-pair-blocked), wv, wo, masks
  q/k projections in fp8-e4m3 DoubleRow (weights pre-scaled x64 on host,
        exp scale absorbs the 1/4096; rel err ~1.7e-2 vs 2e-2 budget)
  qT/kT as [d_g, S] transposed tiles: head-pair t -> partitions
        [0:64] head 2t, [64:128] head 2t+1
  v   [k-tile 128, 8 heads, 65]: col 64 is ones (sumexp lands in the ctx^T
        psum row 64 for free during the attn*V matmul)
  scores^T psum tiles [k 128, 2 heads, q 512] (2 banks): head pair packed
        via PE row tiling (K=64 each, concurrent), one exp over both
  attn = exp(scores/8), causal via skipping k-tiles above the diagonal,
        restricting the q-range on diagonal tiles, and a mask multiply
  ctx^T accumulated in PSUM over k-tiles; normalize via DRAM-roundtrip
        reciprocal + gpsimd partition broadcast (DVE 32x32-transpose
        reciprocal for the last chunk to cut the tail latency).

Schedule: all non-attention matmuls drain as filler INSIDE the attention
stream (interpolated between per-chunk prerequisite markers, 2 k-tiles of
lead); attn*V is emitted one k-tile late so it never head-blocks the FIFO
tensor queue; dummy warm-up matmuls keep/get HAM to full clock during the
initial DMA wait; row-3 out-projection is split so only one matmul + add +
DMA per o-tile remains after the final normalize, with spare out-proj row-2
units retained to keep the PE warm through that normalize.
"""

import numpy as np
import ml_dtypes

import concourse.bass as bass
import concourse.tile as tile
from concourse import bacc, mybir
from concourse.bass_utils import run_bass_kernel_spmd

P = 128          # partitions
S = 2048         # sequence length (one batch per core)
DIN = 1024       # model dim
DG = 512         # head-group width per core (8 heads x 64)
HD = 64          # head dim
NH = 8           # heads per core
QC = 512         # q-chunk (matmul free dim)
NQC = S // QC    # 4 q-chunks
NKT = S // P     # 16 k-tiles
KDT = DIN // P   # 8 din k-tiles
NHP = 4          # head pairs per core
F32 = mybir.dt.float32
BF16 = mybir.dt.bfloat16
FP8 = mybir.dt.float8e4
EXP = mybir.ActivationFunctionType.Exp
DR = mybir.MatmulPerfMode.DoubleRow

USE_FP8_QK = True   # fp8 DoubleRow q/k projections (x64 weight pre-scale)
WSCALE = 64.0
N_WARM = 14         # dummy warm-up matmuls during the initial DMA wait

_CACHE = {}


def _emit(tc, d):
    nc = tc.nc
    with (
        nc.allow_low_precision(reason="bf16 attention pipeline"),
        tc.tile_pool(name="persist", bufs=1) as pp,
        tc.tile_pool(name="work", bufs=4) as wp,
        tc.tile_pool(name="psc", bufs=2, space="PSUM") as psc,
        tc.tile_pool(name="ppj", bufs=2, space="PSUM") as ppj,
        tc.tile_pool(name="pcx", bufs=1, space="PSUM") as pcx,
    ):
        # ---- persistent SBUF tiles (layouts match the pre-tiled DRAM) ----
        xT = pp.tile([P, NQC, KDT, QC], BF16, tag="xT", name="xT")
        if USE_FP8_QK:
            x8 = pp.tile([P, NQC, KDT, QC], FP8, tag="x8", name="x8")
            wq = pp.tile([P, NHP, KDT, P], FP8, tag="wq", name="wq")
            wk = pp.tile([P, NHP, KDT, P], FP8, tag="wk", name="wk")
        else:
            wq = pp.tile([P, NHP, KDT, P], BF16, tag="wq", name="wq")
            wk = pp.tile([P, NHP, KDT, P], BF16, tag="wk", name="wk")
        wv = pp.tile([P, KDT, DG], BF16, tag="wv", name="wv")
        wo = pp.tile([P, 4, DIN], BF16, tag="wo", name="wo")
        qT = [pp.tile([P, S], BF16, tag=f"qT{t}", name=f"qT{t}") for t in range(NHP)]
        kT = [pp.tile([P, S], BF16, tag=f"kT{t}", name=f"kT{t}") for t in range(NHP)]
        vv = [pp.tile([P, NH, HD + 1], BF16, tag=f"v{m}", name=f"v{m}") for m in range(NKT)]
        cx = [pp.tile([P, S], BF16, tag=f"cx{t}", name=f"cx{t}") for t in range(NHP)]
        OB3T = BF16 if USE_FP8_QK else F32
        ob3 = [pp.tile([P, QC], OB3T, tag=f"ob3{o}", name=f"ob3{o}") for o in range(8)]
        msk = pp.tile([P, 4, QC], BF16, tag="msk", name="msk")
        wrm = pp.tile([P, QC], BF16, tag="wrm", name="wrm")

        # ---- PE warm-up: garbage matmuls keep the PE busy (HAM at full
        # clock) while the first input DMAs land ----
        nc.vector.memset(wrm[:], 0.0)
        for g in range(0, N_WARM, 7):
            ps = ppj.tile([P, QC], F32, tag="pj", name="ps")
            n = min(7, N_WARM - g)
            for i in range(n):
                nc.tensor.matmul(
                    ps[:], wrm[:, 0:P], wrm[:],
                    start=(i == 0), stop=(i == n - 1),
                )

        # ---- input DMAs: big contiguous-run transfers, ordered by need,
        # critical prefill set first across both HW-DGE rings ----
        xq = x8 if USE_FP8_QK else xT
        nc.sync.dma_start(xq[:, 0, 0:4, :], d["xq"][:, 0, 0:4, :])
        nc.scalar.dma_start(wq[:, 0, :, :], d["wqT"][:, 0, :, :])
        nc.scalar.dma_start(wk[:, 0, :, :], d["wkT"][:, 0, :, :])
        nc.sync.dma_start(xq[:, 0, 4:KDT, :], d["xq"][:, 0, 4:KDT, :])
        nc.scalar.dma_start(msk[:], d["masks"][:])
        nc.sync.dma_start(wv[:], d["wvT"][:])
        if USE_FP8_QK:
            nc.sync.dma_start(xT[:, 0, :, :], d["xT"][:, 0, :, :])
        nc.scalar.dma_start(wq[:, 1:NHP, :, :], d["wqT"][:, 1:NHP, :, :])
        nc.scalar.dma_start(wk[:, 1:NHP, :, :], d["wkT"][:, 1:NHP, :, :])
        for s in range(1, NQC):
            nc.sync.dma_start(xT[:, s, :, :], d["xT"][:, s, :, :])
            if USE_FP8_QK:
                nc.scalar.dma_start(x8[:, s, :, :], d["xq"][:, s, :, :])
        nc.scalar.dma_start(wo[:], d["woT"][:])

        # ---- filler units ----
        def u_v(m):
            def f():
                ps = ppj.tile([P, QC], F32, tag="pj", name="ps")
                for k in range(KDT):
                    nc.tensor.matmul(
                        ps[:],
                        xT[:, m // 4, k, (m % 4) * P:(m % 4 + 1) * P],
                        wv[:, k, :],
                        start=(k == 0),
                        stop=(k == KDT - 1),
                    )
                nc.vector.tensor_copy(
                    vv[m][:, :, 0:HD], ps[:].rearrange("p (h e) -> p h e", h=NH)
                )
                nc.vector.memset(vv[m][:, :, HD:HD + 1], 1.0)
            return f

        def u_chain(t, w, s):
            def f():
                wt, dst = ((wq, qT), (wk, kT))[w]
                ps = ppj.tile([P, QC], F32, tag="pj", name="ps")
                if USE_FP8_QK:
                    for k in range(0, KDT, 2):
                        nc.tensor.matmul(
                            ps[:],
                            wt[:, t, k:k + 2, :],
                            x8[:, s, k:k + 2, :],
                            start=(k == 0),
                            stop=(k == KDT - 2),
                            perf_mode=DR,
                        )
                else:
                    for k in range(KDT):
                        nc.tensor.matmul(
                            ps[:],
                            wt[:, t, k, :],
                            xT[:, s, k, :],
                            start=(k == 0),
                            stop=(k == KDT - 1),
                        )
                nc.vector.tensor_copy(dst[t][:, s * QC:(s + 1) * QC], ps[:])
            return f

        def u_out(s, o):
            def f():
                ps = ppj.tile([P, QC], F32, tag="pj", name="ps")
                for k in range(4):
                    nc.tensor.matmul(
                        ps[:],
                        wo[:, k, o * P:(o + 1) * P],
                        cx[k][:, s * QC:(s + 1) * QC],
                        start=(k == 0), stop=(k == 3),
                    )
                ob = wp.tile([P, QC], F32, tag="ob", name="ob", bufs=2)
                nc.vector.tensor_copy(ob[:], ps[:])
                nc.sync.dma_start(
                    d["outT"][o * P:(o + 1) * P, s * QC:(s + 1) * QC], ob[:]
                )
            return f

        def u_out3_partial(o):
            # row-3 out-proj, head-pair groups 0..2 only -> SBUF partial
            def f():
                ps = ppj.tile([P, QC], F32, tag="pj", name="ps")
                for k in range(3):
                    nc.tensor.matmul(
                        ps[:],
                        wo[:, k, o * P:(o + 1) * P],
                        cx[k][:, 3 * QC:S],
                        start=(k == 0), stop=(k == 2),
                    )
                nc.vector.tensor_copy(ob3[o][:], ps[:])
            return f

        def u_out3_final(o):
            ps = ppj.tile([P, QC], F32, tag="pj", name="ps")
            nc.tensor.matmul(
                ps[:], wo[:, 3, o * P:(o + 1) * P], cx[3][:, 3 * QC:S],
                start=True, stop=True,
            )
            ob = wp.tile([P, QC], F32, tag="ob", name="ob", bufs=2)
            nc.vector.tensor_add(ob[:], ps[:], ob3[o][:])
            nc.sync.dma_start(d["outT"][o * P:(o + 1) * P, 3 * QC:S], ob[:])

        # consume-ordered filler queue + hard prerequisites per chunk
        queue = [u_v(0), u_v(1), u_v(2), u_v(3)]
        pre = {}
        for s in range(NQC):
            for hp in range(NHP):
                if (hp, s) == (0, 0):
                    pre[(hp, s)] = 0
                    continue
                if hp == 0 and s >= 1:
                    queue += [u_v(m) for m in range(4 * s, 4 * s + 4)]
                queue += [u_chain(hp, 0, s), u_chain(hp, 1, s)]
                pre[(hp, s)] = len(queue)
            if s == 1 or s == 2:
                queue += [u_out(s - 1, o) for o in range(8)]
        queue += [u_out(2, o) for o in range(8)]
        queue += [u_out3_partial(o) for o in range(8)]
        n_units = len(queue)

        order = [(hp, s) for s in range(NQC) for hp in range(NHP)]
        nxt = {order[i]: order[i + 1] for i in range(len(order) - 1)}

        state = {"drained": 0}

        def drain_to(idx):
            while state["drained"] < idx:
                queue[state["drained"]]()
                state["drained"] += 1

        def normalize(hp, s):
            last = (hp, s) == (NHP - 1, NQC - 1)
            cb = wp.tile([96, 2, QC], F32, tag="cb", name="cb", bufs=2)
            cps = state["cps"]
            nc.vector.tensor_copy(cb[0:HD + 1], cps[:])
            if last:
                # low-latency path: DVE 32x32 block-transpose reshapes the
                # [1,1024] sumexp row (row 0 of the 32-aligned cb[64:96]
                # window; rows 65:96 are filler) onto 32 partitions,
                # iterative reciprocal on free-dim 32, transpose back -
                # no SBUF-DMA roundtrips in the tail
                nc.vector.tensor_copy(cb[64:96], cps[0:32])
                nc.vector.tensor_copy(cb[64:65], cps[HD:HD + 1])
                t1 = wp.tile([32, 2, QC], F32, tag="t1", name="t1", bufs=1)
                nc.vector.transpose(t1[:], cb[64:96])
                t2 = wp.tile([32, 2, QC], F32, tag="t2", name="t2", bufs=1)
                nc.vector.tensor_copy(t2[:], t1[:])
                tv1 = t1[:].rearrange("p h (b j) -> p (h b) j", j=32)
                tv2 = t2[:].rearrange("p h (b j) -> p (h b) j", j=32)
                nc.vector.reciprocal(tv2[:, :, 0:1], tv1[:, :, 0:1])
                rc = wp.tile([32, 2, QC], F32, tag="rc", name="rc", bufs=2)
                nc.vector.transpose(rc[:], t2[:])
                rrow = rc[0:1, :, :]
            else:
                zt = wp.tile([P, 8], F32, tag="zt", name="zt", bufs=2)
                nc.sync.dma_start(zt[:], cb[HD:HD + 1, :, :])
                rt = wp.tile([P, 8], F32, tag="rt", name="rt", bufs=2)
                nc.vector.reciprocal(rt[:], zt[:])
                rc = wp.tile([32, 2, QC], F32, tag="rc", name="rc", bufs=2)
                nc.sync.dma_start(rc[0:1, :, :], rt[:])
                rrow = rc[0:1, :, :]
            bs = wp.tile([HD, 2, QC], F32, tag="bs", name="bs", bufs=2)
            nc.gpsimd.partition_broadcast(bs[:], rrow)
            # head B first: its partition-shift DMA overlaps head A's mul
            cxs = wp.tile([HD, QC], BF16, tag="cxs", name="cxs", bufs=2)
            nc.vector.tensor_mul(cxs[:], cb[0:HD, 1, :], bs[:, 1, :])
            nc.sync.dma_start(cx[hp][HD:P, s * QC:(s + 1) * QC], cxs[:])
            nc.vector.tensor_mul(
                cx[hp][0:HD, s * QC:(s + 1) * QC], cb[0:HD, 0, :], bs[:, 0, :]
            )

        def attn_chunk(hp, s):
            t0 = pre[(hp, s)]
            t1 = pre[nxt[(hp, s)]] if (hp, s) in nxt else n_units
            nkt = 4 * (s + 1)  # causal: k-tiles 0..nkt-1
            cps = pcx.tile([HD + 1, 2, QC], F32, tag="cx", name="cps")
            state["cps"] = cps

            def attn_v(k, s0, a):
                nc.tensor.matmul(
                    cps[:, 0, s0:], vv[k][:, 2 * hp, :], a[:, 0, s0:],
                    start=(k == 0), stop=(k == nkt - 1),
                )
                nc.tensor.matmul(
                    cps[:, 1, s0:], vv[k][:, 2 * hp + 1, :], a[:, 1, s0:],
                    start=(k == 0), stop=(k == nkt - 1),
                )

            pend = None  # attn*V emitted one k-tile late: by the time it
            # reaches the head of the FIFO tensor queue its exp is done, so
            # it never head-blocks the scores stream behind it
            for k in range(nkt):
                dd = k - 4 * s
                s0 = max(dd, 0) * P  # causal q-range restriction
                sps = psc.tile([P, 2, QC], F32, tag="sc", name="sps")
                nc.tensor.matmul(
                    sps[:, 0, s0:],
                    kT[hp][0:HD, k * P:(k + 1) * P],
                    qT[hp][0:HD, s * QC + s0:(s + 1) * QC],
                    start=True, stop=True,
                )
                nc.tensor.matmul(
                    sps[:, 1, s0:],
                    kT[hp][HD:P, k * P:(k + 1) * P],
                    qT[hp][HD:P, s * QC + s0:(s + 1) * QC],
                    start=True, stop=True,
                )
                a = wp.tile([P, 2, QC], BF16, tag="a", name="a", bufs=5)
                nc.scalar.activation(
                    a[:, :, s0:], sps[:, :, s0:], EXP, scale=d["escale"]
                )
                if dd >= 0:
                    # only columns [s0, s0+128) straddle the diagonal
                    for h in range(2):
                        nc.vector.tensor_mul(
                            a[:, h, s0:s0 + P], a[:, h, s0:s0 + P],
                            msk[:, dd, s0:s0 + P],
                        )
                if pend is not None:
                    attn_v(*pend)
                pend = (k, s0, a)
                drain_to(min(t1, t0 + ((t1 - t0) * (k + 3)) // nkt,
                             state["drained"] + 2))
            attn_v(*pend)
            normalize(hp, s)

        # ---- prefill: just enough to start chunk (0,0) ----
        u_chain(0, 0, 0)()
        u_chain(0, 1, 0)()

        # ---- main stream ----
        for s in range(NQC):
            for hp in range(NHP):
                drain_to(pre[(hp, s)])
                attn_chunk(hp, s)
        drain_to(n_units)
        for o in range(8):
            u_out3_final(o)


def _build():
    if "nc" in _CACHE:
        return _CACHE["nc"]
    nc = bacc.Bacc("TRN2", target_bir_lowering=False, debug=False, num_devices=8)
    d = {
        "xT": nc.dram_tensor("xT", [P, NQC, KDT, QC], BF16, kind="ExternalInput").ap(),
        "wvT": nc.dram_tensor("wvT", [P, KDT, DG], BF16, kind="ExternalInput").ap(),
        "woT": nc.dram_tensor("woT", [P, 4, DIN], BF16, kind="ExternalInput").ap(),
        "masks": nc.dram_tensor("masks", [P, 4, QC], BF16, kind="ExternalInput").ap(),
        "outT": nc.dram_tensor("outT", [DIN, S], F32, kind="ExternalOutput").ap(),
    }
    wdt = FP8 if USE_FP8_QK else BF16
    d["wqT"] = nc.dram_tensor("wqT", [P, NHP, KDT, P], wdt, kind="ExternalInput").ap()
    d["wkT"] = nc.dram_tensor("wkT", [P, NHP, KDT, P], wdt, kind="ExternalInput").ap()
    if USE_FP8_QK:
        d["xq"] = nc.dram_tensor("xq", [P, NQC, KDT, QC], FP8, kind="ExternalInput").ap()
        d["escale"] = 0.125 / (WSCALE * WSCALE)
    else:
        d["xq"] = d["xT"]
        d["escale"] = 0.125
    with tile.TileContext(nc) as tc:
        _emit(tc, d)
    nc.compile()
    _CACHE["nc"] = nc
    return nc


def _masks_np():
    r = np.arange(P)[:, None]
    j = np.arange(QC)[None, :]
    m = np.stack(
        [(j >= r + dd * P).astype(ml_dtypes.bfloat16) for dd in range(4)], axis=1
    )  # [128, 4, 512]
    return np.ascontiguousarray(m)


def _tile_k(a, kdt=KDT):
    """[kdt*P, C] -> [P, kdt, C] (din-subtile blocking)."""
    c = a.shape[1]
    return np.ascontiguousarray(a.reshape(kdt, P, c).transpose(1, 0, 2))


def _f8(a):
    return np.clip(a, -240, 240).astype(ml_dtypes.float8_e4m3)


def kernel(x, Wq, Wk, Wv, Wo, bo, _run_kwargs=None, _return_res=False):
    x = np.asarray(x)
    Wq, Wk, Wv, Wo, bo = (np.asarray(a) for a in (Wq, Wk, Wv, Wo, bo))
    B = x.shape[0]
    nc = _build()

    def b16(a):
        return np.ascontiguousarray(a).astype(ml_dtypes.bfloat16)

    masks = _masks_np()
    in_maps = []
    for c in range(8):
        b, g = divmod(c, 2)
        xt = b16(x[b].T)  # [1024, 2048]
        xt4 = xt.reshape(KDT, P, NQC, QC).transpose(1, 2, 0, 3)  # [p,s,k,c]
        wqt = Wq[g * DG:(g + 1) * DG, :].T  # [1024, 512] f32
        wkt = Wk[g * DG:(g + 1) * DG, :].T
        im = {
            "xT": np.ascontiguousarray(xt4),
            "wvT": _tile_k(b16(Wv[g * DG:(g + 1) * DG, :].T)),
            "woT": _tile_k(b16(Wo[:, g * DG:(g + 1) * DG].T), kdt=4),
            "masks": masks,
        }
        if USE_FP8_QK:
            im["xq"] = np.ascontiguousarray(
                _f8(x[b].T).reshape(KDT, P, NQC, QC).transpose(1, 2, 0, 3))
            im["wqT"] = np.ascontiguousarray(
                _f8(WSCALE * wqt).reshape(KDT, P, NHP, P).transpose(1, 2, 0, 3))
            im["wkT"] = np.ascontiguousarray(
                _f8(WSCALE * wkt).reshape(KDT, P, NHP, P).transpose(1, 2, 0, 3))
        else:
            im["wqT"] = np.ascontiguousarray(
                b16(wqt).reshape(KDT, P, NHP, P).transpose(1, 2, 0, 3))
            im["wkT"] = np.ascontiguousarray(
                b16(wkt).reshape(KDT, P, NHP, P).transpose(1, 2, 0, 3))
        in_maps.append(im)

    res = run_bass_kernel_spmd(nc, in_maps, list(range(8)), **(_run_kwargs or {}))
    out = np.empty((B, S, DIN), np.float32)
    for b in range(B):
        p = res.results[2 * b]["outT"] + res.results[2 * b + 1]["outT"]
        out[b] = p.T + bo.astype(np.float32)
    if _return_res:
        return out, res
    return out



# revision 7
# speedup vs baseline: 1.0616x; 1.0616x over previous
"""Causal multi-head attention (B=4, S=2048, D=1024, H=16, hd=64) on 8 TRN2
NeuronCores.

Sharding: core c = (batch b = c//2, head-group g = c%2). Each core computes
QKV projections for its 8 heads (Megatron column-split), causal attention,
and a partial out-projection (row-split); the host sums the two head-group
partials per batch and adds the bias.

On-device layout (bf16 compute, fp32 PSUM accumulation):
  xT  [p, q-block, din-subtile, 512]  x[b]^T pre-tiled on host so each
        input DMA moves 8KB-contiguous runs per partition (descriptor-
        efficient); same for wq/wk (head-pair-blocked), wv, wo
  q/k projections in fp8-e4m3 DoubleRow (weights pre-scaled x64 on host,
        the exp scale absorbs the 1/4096; value path stays bf16 - fp8
        anywhere in v/attn costs ~1% extra rel err, over budget)
  qT/kT as [d_g, S] transposed tiles: head-pair t -> partitions
        [0:64] head 2t, [64:128] head 2t+1
  v   [k-tile 128, 8 heads, 65]: col 64 is ones (sumexp lands in the ctx^T
        psum row 64 for free during the attn*V matmul)
  scores^T psum tiles [k 128, 2 heads, q 512]: head pair packed via PE row
        tiling (K=64 each, concurrent).  Scores for TWO k-tiles are emitted
        back-to-back: full-array<->row-group LDWEIGHTS transitions stall
        ~100ns each (the PE can only pull an LDW ahead of in-flight matmuls
        into a non-conflicting row group), so batching the row-tiled pairs
        halves the number of transitions.
  attn = exp(scores/8) per k-tile on ScalarE; causal via skipping k-tiles
        above the diagonal, restricting the q-range on diagonal tiles, and
        one masked multiply per diagonal tile (the 128-wide diagonal block
        is the same upper triangle for every dd, both heads in one op)
  ctx^T accumulated in PSUM over k-tiles; normalize via DRAM-roundtrip
        reciprocal + gpsimd partition broadcast.  The last chunk instead
        transposes the sumexp row straight out of PSUM (DVE 32x32 block
        transpose), reciprocates in place, transposes back, and multiplies
        straight from PSUM - no SBUF copies on the critical tail.

Schedule: all non-attention matmuls drain as filler INSIDE the attention
stream (interpolated between per-chunk prerequisite markers); attn*V is
emitted one k-tile-PAIR late so its exp is always ready; ~10 short dummy
matmuls keep HAM at full clock through the initial DMA wait without
head-blocking the prefill; input DMAs are spread over the sync, scalar and
gpsimd rings so the scalar queue is clear before the exp stream starts;
row-3 out-projection is split so only one matmul + add + DMA per o-tile
remains after the final normalize, pipelined over psum/sbuf double-buffers
and both DMA rings.
"""

import numpy as np
import ml_dtypes

import concourse.bass as bass
import concourse.tile as tile
from concourse import bacc, mybir
from concourse.bass_utils import run_bass_kernel_spmd

P = 128          # partitions
S = 2048         # sequence length (one batch per core)
DIN = 1024       # model dim
DG = 512         # head-group width per core (8 heads x 64)
HD = 64          # head dim
NH = 8           # heads per core
QC = 512         # q-chunk (matmul free dim)
NQC = S // QC    # 4 q-chunks
NKT = S // P     # 16 k-tiles
KDT = DIN // P   # 8 din k-tiles
NHP = 4          # head pairs per core
F32 = mybir.dt.float32
BF16 = mybir.dt.bfloat16
FP8 = mybir.dt.float8e4
EXP = mybir.ActivationFunctionType.Exp
DR = mybir.MatmulPerfMode.DoubleRow

WSCALE = 64.0    # fp8 q/k weight pre-scale
N_WARM = 10      # dummy warm-up matmuls (N=256) during the initial DMA wait
WARM_N = 256

_CACHE = {}


def _emit(tc, d):
    nc = tc.nc
    with (
        nc.allow_low_precision(reason="bf16 attention pipeline"),
        tc.tile_pool(name="persist", bufs=1) as pp,
        tc.tile_pool(name="work", bufs=4) as wp,
        tc.tile_pool(name="psc", bufs=2, space="PSUM") as psc,
        tc.tile_pool(name="ppj", bufs=2, space="PSUM") as ppj,
        tc.tile_pool(name="pcx", bufs=1, space="PSUM") as pcx,
    ):
        # ---- persistent SBUF tiles (layouts match the pre-tiled DRAM) ----
        xT = pp.tile([P, NQC, KDT, QC], BF16, tag="xT", name="xT")
        x8 = pp.tile([P, NQC, KDT, QC], FP8, tag="x8", name="x8")
        wq = pp.tile([P, NHP, KDT, P], FP8, tag="wq", name="wq")
        wk = pp.tile([P, NHP, KDT, P], FP8, tag="wk", name="wk")
        wv = pp.tile([P, KDT, DG], BF16, tag="wv", name="wv")
        wo = pp.tile([P, 4, DIN], BF16, tag="wo", name="wo")
        qT = [pp.tile([P, S], BF16, tag=f"qT{t}", name=f"qT{t}") for t in range(NHP)]
        kT = [pp.tile([P, S], BF16, tag=f"kT{t}", name=f"kT{t}") for t in range(NHP)]
        vv = [pp.tile([P, NH, HD + 1], BF16, tag=f"v{m}", name=f"v{m}") for m in range(NKT)]
        cx = [pp.tile([P, S], BF16, tag=f"cx{t}", name=f"cx{t}") for t in range(NHP)]
        ob3 = [pp.tile([P, QC], BF16, tag=f"ob3{o}", name=f"ob3{o}") for o in range(8)]
        msk = pp.tile([P, 2, P], BF16, tag="msk", name="msk")
        wrm = pp.tile([P, WARM_N], BF16, tag="wrm", name="wrm")

        # ---- PE warm-up: short garbage matmuls keep the PE busy (HAM at
        # full clock) while the first input DMAs land, without committing
        # the PE FIFO much past the x8 arrival ----
        nc.vector.memset(wrm[:], 0.0)
        for g in range(0, N_WARM, 5):
            ps = ppj.tile([P, WARM_N], F32, tag="pj", name="ps")
            n = min(5, N_WARM - g)
            for i in range(n):
                nc.tensor.matmul(
                    ps[:], wrm[:, 0:P], wrm[:],
                    start=(i == 0), stop=(i == n - 1),
                )

        # ---- input DMAs: big contiguous-run transfers, ordered by need,
        # critical prefill set first, spread across three HW-DGE rings so
        # the scalar queue is idle before the exp stream starts ----
        nc.sync.dma_start(x8[:, 0, 0:4, :], d["xq"][:, 0, 0:4, :])
        nc.scalar.dma_start(wq[:, 0, :, :], d["wqT"][:, 0, :, :])
        nc.scalar.dma_start(wk[:, 0, :, :], d["wkT"][:, 0, :, :])
        nc.sync.dma_start(x8[:, 0, 4:KDT, :], d["xq"][:, 0, 4:KDT, :])
        nc.scalar.dma_start(msk[:], d["masks"][:])
        nc.sync.dma_start(wv[:], d["wvT"][:])
        nc.sync.dma_start(xT[:, 0, :, :], d["xT"][:, 0, :, :])
        nc.scalar.dma_start(wq[:, 1:NHP, :, :], d["wqT"][:, 1:NHP, :, :])
        nc.scalar.dma_start(wk[:, 1:NHP, :, :], d["wkT"][:, 1:NHP, :, :])
        for s in range(1, NQC):
            nc.sync.dma_start(xT[:, s, :, :], d["xT"][:, s, :, :])
            nc.gpsimd.dma_start(x8[:, s, :, :], d["xq"][:, s, :, :])
        nc.gpsimd.dma_start(wo[:], d["woT"][:])

        # ---- filler units ----
        def u_v(m):
            def f():
                ps = ppj.tile([P, QC], F32, tag="pj", name="ps")
                for k in range(KDT):
                    nc.tensor.matmul(
                        ps[:],
                        xT[:, m // 4, k, (m % 4) * P:(m % 4 + 1) * P],
                        wv[:, k, :],
                        start=(k == 0),
                        stop=(k == KDT - 1),
                    )
                nc.vector.tensor_copy(
                    vv[m][:, :, 0:HD], ps[:].rearrange("p (h e) -> p h e", h=NH)
                )
                nc.vector.memset(vv[m][:, :, HD:HD + 1], 1.0)
            return f

        def u_chain(t, w, s):
            def f():
                wt, dst = ((wq, qT), (wk, kT))[w]
                ps = ppj.tile([P, QC], F32, tag="pj", name="ps")
                for k in range(0, KDT, 2):
                    nc.tensor.matmul(
                        ps[:],
                        wt[:, t, k:k + 2, :],
                        x8[:, s, k:k + 2, :],
                        start=(k == 0),
                        stop=(k == KDT - 2),
                        perf_mode=DR,
                    )
                nc.vector.tensor_copy(dst[t][:, s * QC:(s + 1) * QC], ps[:])
            return f

        def u_out(s, o):
            def f():
                ps = ppj.tile([P, QC], F32, tag="pj", name="ps")
                for k in range(4):
                    nc.tensor.matmul(
                        ps[:],
                        wo[:, k, o * P:(o + 1) * P],
                        cx[k][:, s * QC:(s + 1) * QC],
                        start=(k == 0), stop=(k == 3),
                    )
                ob = wp.tile([P, QC], F32, tag="ob", name="ob", bufs=2)
                nc.vector.tensor_copy(ob[:], ps[:])
                nc.sync.dma_start(
                    d["outT"][o * P:(o + 1) * P, s * QC:(s + 1) * QC], ob[:]
                )
            return f

        def u_out3_partial(o):
            # row-3 out-proj, head-pair groups 0..2 only -> SBUF partial
            def f():
                ps = ppj.tile([P, QC], F32, tag="pj", name="ps")
                for k in range(3):
                    nc.tensor.matmul(
                        ps[:],
                        wo[:, k, o * P:(o + 1) * P],
                        cx[k][:, 3 * QC:S],
                        start=(k == 0), stop=(k == 2),
                    )
                nc.vector.tensor_copy(ob3[o][:], ps[:])
            return f

        def u_out3_final(o):
            ps = ppj.tile([P, QC], F32, tag="pj", name="ps")
            nc.tensor.matmul(
                ps[:], wo[:, 3, o * P:(o + 1) * P], cx[3][:, 3 * QC:S],
                start=True, stop=True,
            )
            ob = wp.tile([P, QC], F32, tag="obf", name="obf", bufs=4)
            nc.vector.tensor_add(ob[:], ps[:], ob3[o][:])
            eng = nc.sync if o % 2 == 0 else nc.gpsimd
            eng.dma_start(d["outT"][o * P:(o + 1) * P, 3 * QC:S], ob[:])

        # consume-ordered filler queue + hard prerequisites per chunk
        queue = [u_v(0), u_v(1), u_v(2), u_v(3)]
        pre = {}
        for s in range(NQC):
            for hp in range(NHP):
                if (hp, s) == (0, 0):
                    pre[(hp, s)] = 0
                    continue
                if hp == 0 and s >= 1:
                    queue += [u_v(m) for m in range(4 * s, 4 * s + 4)]
                queue += [u_chain(hp, 0, s), u_chain(hp, 1, s)]
                pre[(hp, s)] = len(queue)
            if s == 1 or s == 2:
                queue += [u_out(s - 1, o) for o in range(8)]
        queue += [u_out(2, o) for o in range(8)]
        queue += [u_out3_partial(o) for o in range(8)]
        n_units = len(queue)

        order = [(hp, s) for s in range(NQC) for hp in range(NHP)]
        nxt = {order[i]: order[i + 1] for i in range(len(order) - 1)}

        state = {"drained": 0}

        def drain_to(idx):
            while state["drained"] < idx:
                queue[state["drained"]]()
                state["drained"] += 1

        def normalize(hp, s):
            last = (hp, s) == (NHP - 1, NQC - 1)
            cps = state["cps"]
            if last:
                # low-latency tail: DVE 32x32 block-transpose lifts the
                # [1,1024] sumexp row (row 64 = col 0 of the 32-aligned psum
                # window [64:96]; rows 65:96 are memset filler) onto 32
                # partitions straight out of PSUM, reciprocal in place
                # (col 0 -> col 1), transpose back, then multiply straight
                # from PSUM - no SBUF staging on the critical tail
                t1 = wp.tile([32, 2, QC], F32, tag="t1", name="t1", bufs=1)
                t2 = wp.tile([32, 2, QC], F32, tag="t2", name="t2", bufs=1)
                nc.vector.memset(t2[:], 1.0)  # early, off the critical path
                nc.vector.transpose(t1[:], cps[64:96, :, :])
                tv1 = t1[:].rearrange("p h (b j) -> p h b j", j=32)
                tv2 = t2[:].rearrange("p h (b j) -> p h b j", j=32)
                nc.vector.reciprocal(tv2[:, :, :, 0:1], tv1[:, :, :, 0:1])
                rc = wp.tile([32, 2, QC], F32, tag="rc", name="rc", bufs=1)
                nc.vector.transpose(rc[:], t2[:])
                bs = wp.tile([HD, 2, QC], F32, tag="bs", name="bs", bufs=2)
                nc.gpsimd.partition_broadcast(bs[:], rc[0:1, :, :])
                cxs = wp.tile([HD, QC], BF16, tag="cxs", name="cxs", bufs=2)
                nc.vector.tensor_mul(cxs[:], cps[0:HD, 1, :], bs[:, 1, :])
                nc.sync.dma_start(cx[hp][HD:P, s * QC:(s + 1) * QC], cxs[:])
                nc.vector.tensor_mul(
                    cx[hp][0:HD, s * QC:(s + 1) * QC], cps[0:HD, 0, :], bs[:, 0, :]
                )
                return
            cb = wp.tile([HD + 1, 2, QC], F32, tag="cb", name="cb", bufs=2)
            nc.vector.tensor_copy(cb[:], cps[0:HD + 1, :, :])
            zt = wp.tile([P, 8], F32, tag="zt", name="zt", bufs=2)
            nc.sync.dma_start(zt[:], cb[HD:HD + 1, :, :])
            rt = wp.tile([P, 8], F32, tag="rt", name="rt", bufs=2)
            nc.vector.reciprocal(rt[:], zt[:])
            rr = wp.tile([1, 2, QC], F32, tag="rr", name="rr", bufs=2)
            nc.sync.dma_start(rr[:], rt[:])
            bs = wp.tile([HD, 2, QC], F32, tag="bs", name="bs", bufs=2)
            nc.gpsimd.partition_broadcast(bs[:], rr[:])
            # head B first: its partition-shift DMA overlaps head A's mul
            cxs = wp.tile([HD, QC], BF16, tag="cxs", name="cxs", bufs=2)
            nc.vector.tensor_mul(cxs[:], cb[0:HD, 1, :], bs[:, 1, :])
            nc.sync.dma_start(cx[hp][HD:P, s * QC:(s + 1) * QC], cxs[:])
            nc.vector.tensor_mul(
                cx[hp][0:HD, s * QC:(s + 1) * QC], cb[0:HD, 0, :], bs[:, 0, :]
            )

        def attn_chunk(hp, s):
            t0 = pre[(hp, s)]
            t1 = pre[nxt[(hp, s)]] if (hp, s) in nxt else n_units
            nkt = 4 * (s + 1)  # causal: k-tiles 0..nkt-1
            cps = pcx.tile([96, 2, QC], F32, tag="cx", name="cps")
            state["cps"] = cps
            if (hp, s) == (NHP - 1, NQC - 1):
                # valid filler above the sumexp row for the tail transpose
                # (32-aligned window; row 64 is re-written by the start=True
                # attn*V accumulation right after)
                nc.vector.memset(cps[HD:96, :, :], 1.0)

            def attn_v_pair(pair):
                for k, s0, a in pair:
                    nc.tensor.matmul(
                        cps[0:HD + 1, 0, s0:], vv[k][:, 2 * hp, :], a[:, 0, s0:],
                        start=(k == 0), stop=(k == nkt - 1),
                    )
                    nc.tensor.matmul(
                        cps[0:HD + 1, 1, s0:], vv[k][:, 2 * hp + 1, :], a[:, 1, s0:],
                        start=(k == 0), stop=(k == nkt - 1),
                    )

            pend = None  # attn*V emitted one k-tile-PAIR late: its exp and
            # mask are always done by the time it reaches the head of the
            # FIFO tensor queue, so it never head-blocks the scores stream
            for pk in range(nkt // 2):
                tiles = []
                # scores for both k-tiles of the pair back-to-back: keeps
                # the row-tiled LDWEIGHTS adjacent (one full-array<->row-
                # group transition per pair instead of two)
                for k in (2 * pk, 2 * pk + 1):
                    dd = k - 4 * s
                    s0 = max(dd, 0) * P  # causal q-range restriction
                    sps = psc.tile([P, 2, QC], F32, tag="sc", name="sps")
                    nc.tensor.matmul(
                        sps[:, 0, s0:],
                        kT[hp][0:HD, k * P:(k + 1) * P],
                        qT[hp][0:HD, s * QC + s0:(s + 1) * QC],
                        start=True, stop=True,
                    )
                    nc.tensor.matmul(
                        sps[:, 1, s0:],
                        kT[hp][HD:P, k * P:(k + 1) * P],
                        qT[hp][HD:P, s * QC + s0:(s + 1) * QC],
                        start=True, stop=True,
                    )
                    tiles.append((k, s0, sps))
                cur = []
                for k, s0, sps in tiles:
                    a = wp.tile([P, 2, QC], BF16, tag="a", name="a", bufs=5)
                    nc.scalar.activation(
                        a[:, :, s0:], sps[:, :, s0:], EXP, scale=d["escale"]
                    )
                    if k - 4 * s >= 0:
                        # only columns [s0, s0+128) straddle the diagonal;
                        # the block mask is the same triangle for every dd
                        nc.vector.tensor_mul(
                            a[:, :, s0:s0 + P], a[:, :, s0:s0 + P], msk[:]
                        )
                    cur.append((k, s0, a))
                if pend is not None:
                    attn_v_pair(pend)
                pend = cur
                drain_to(min(t1, t0 + ((t1 - t0) * (2 * pk + 4)) // nkt,
                             state["drained"] + 4))
            attn_v_pair(pend)
            normalize(hp, s)

        # ---- prefill: just enough to start chunk (0,0) ----
        u_chain(0, 0, 0)()
        u_chain(0, 1, 0)()

        # ---- main stream ----
        for s in range(NQC):
            for hp in range(NHP):
                drain_to(pre[(hp, s)])
                attn_chunk(hp, s)
        drain_to(n_units)
        for o in range(8):
            u_out3_final(o)


def _build():
    if "nc" in _CACHE:
        return _CACHE["nc"]
    nc = bacc.Bacc("TRN2", target_bir_lowering=False, debug=False, num_devices=8)
    d = {
        "xT": nc.dram_tensor("xT", [P, NQC, KDT, QC], BF16, kind="ExternalInput").ap(),
        "wvT": nc.dram_tensor("wvT", [P, KDT, DG], BF16, kind="ExternalInput").ap(),
        "woT": nc.dram_tensor("woT", [P, 4, DIN], BF16, kind="ExternalInput").ap(),
        "masks": nc.dram_tensor("masks", [P, 2, P], BF16, kind="ExternalInput").ap(),
        "outT": nc.dram_tensor("outT", [DIN, S], F32, kind="ExternalOutput").ap(),
        "wqT": nc.dram_tensor("wqT", [P, NHP, KDT, P], FP8, kind="ExternalInput").ap(),
        "wkT": nc.dram_tensor("wkT", [P, NHP, KDT, P], FP8, kind="ExternalInput").ap(),
        "xq": nc.dram_tensor("xq", [P, NQC, KDT, QC], FP8, kind="ExternalInput").ap(),
        "escale": 0.125 / (WSCALE * WSCALE),
    }
    with tile.TileContext(nc) as tc:
        _emit(tc, d)
    nc.compile()
    _CACHE["nc"] = nc
    return nc


def _masks_np():
    r = np.arange(P)[:, None]
    j = np.arange(P)[None, :]
    m = (j >= r).astype(ml_dtypes.bfloat16)  # [128, 128] upper triangle
    return np.ascontiguousarray(np.broadcast_to(m[:, None, :], (P, 2, P)))


def _tile_k(a, kdt=KDT):
    """[kdt*P, C] -> [P, kdt, C] (din-subtile blocking)."""
    c = a.shape[1]
    return np.ascontiguousarray(a.reshape(kdt, P, c).transpose(1, 0, 2))


def _f8(a):
    return np.clip(a, -240, 240).astype(ml_dtypes.float8_e4m3)


def kernel(x, Wq, Wk, Wv, Wo, bo, _run_kwargs=None, _return_res=False):
    x = np.asarray(x)
    Wq, Wk, Wv, Wo, bo = (np.asarray(a) for a in (Wq, Wk, Wv, Wo, bo))
    B = x.shape[0]
    nc = _build()

    def b16(a):
        return np.ascontiguousarray(a).astype(ml_dtypes.bfloat16)

    masks = _masks_np()
    in_maps = []
    for c in range(8):
        b, g = divmod(c, 2)
        xt = b16(x[b].T)  # [1024, 2048]
        xt4 = xt.reshape(KDT, P, NQC, QC).transpose(1, 2, 0, 3)  # [p,s,k,c]
        wqt = Wq[g * DG:(g + 1) * DG, :].T  # [1024, 512] f32
        wkt = Wk[g * DG:(g + 1) * DG, :].T
        im = {
            "xT": np.ascontiguousarray(xt4),
            "wvT": _tile_k(b16(Wv[g * DG:(g + 1) * DG, :].T)),
            "woT": _tile_k(b16(Wo[:, g * DG:(g + 1) * DG].T), kdt=4),
            "masks": masks,
            "xq": np.ascontiguousarray(
                _f8(x[b].T).reshape(KDT, P, NQC, QC).transpose(1, 2, 0, 3)),
            "wqT": np.ascontiguousarray(
                _f8(WSCALE * wqt).reshape(KDT, P, NHP, P).transpose(1, 2, 0, 3)),
            "wkT": np.ascontiguousarray(
                _f8(WSCALE * wkt).reshape(KDT, P, NHP, P).transpose(1, 2, 0, 3)),
        }
        in_maps.append(im)

    res = run_bass_kernel_spmd(nc, in_maps, list(range(8)), **(_run_kwargs or {}))
    out = np.empty((B, S, DIN), np.float32)
    for b in range(B):
        p = res.results[2 * b]["outT"] + res.results[2 * b + 1]["outT"]
        out[b] = p.T + bo.astype(np.float32)
    if _return_res:
        return out, res
    return out


# revision 17
# speedup vs baseline: 1.0695x; 1.0074x over previous
"""Causal multi-head attention (B=4, S=2048, D=1024, H=16, hd=64) on 8 TRN2
NeuronCores.

Sharding: core c = (batch b = c//2, head-group g = c%2). Each core computes
QKV projections for its 8 heads (Megatron column-split), causal attention,
and a partial out-projection (row-split); the host sums the two head-group
partials per batch and adds the bias.

On-device layout (bf16 compute, fp32 PSUM accumulation):
  xT  [p, q-block, din-subtile, 512]  x[b]^T pre-tiled on host so each
        input DMA moves 8KB-contiguous runs per partition (descriptor-
        efficient); same for wq/wk (head-pair-blocked), wv, wo
  q/k projections in fp8-e4m3 DoubleRow (weights pre-scaled x64 on host,
        the exp scale absorbs the 1/4096; value path stays bf16 - fp8
        anywhere in v/attn costs ~1% extra rel err, over budget)
  qT/kT as [d_g, S] transposed tiles: head-pair t -> partitions
        [0:64] head 2t, [64:128] head 2t+1
  v   [k-tile 128, 8 heads, 65]: col 64 is ones (sumexp lands in the ctx^T
        psum row 64 for free during the attn*V matmul)
  scores^T psum tiles [k 128, 2 heads, q 512]: head pair packed via PE row
        tiling (K=64 each, concurrent).  Scores for TWO k-tiles are emitted
        back-to-back: full-array<->row-group LDWEIGHTS transitions stall
        ~100ns each (the PE can only pull an LDW ahead of in-flight matmuls
        into a non-conflicting row group), so batching the row-tiled pairs
        halves the number of transitions.
  attn = exp(scores/8) per k-tile on ScalarE; causal via skipping k-tiles
        above the diagonal, restricting the q-range on diagonal tiles, and
        one masked multiply per diagonal tile (the 128-wide diagonal block
        is the same upper triangle for every dd, both heads in one op)
  ctx^T accumulated in PSUM over k-tiles; normalize via DRAM-roundtrip
        reciprocal + gpsimd partition broadcast.  The last chunk instead
        transposes the sumexp row straight out of PSUM (DVE 32x32 block
        transpose), reciprocates in place, transposes back, and multiplies
        straight from PSUM - no SBUF copies on the critical tail.

Schedule: all non-attention matmuls drain as filler INSIDE the attention
stream (interpolated between per-chunk prerequisite markers); attn*V is
emitted one k-tile-PAIR late so its exp is always ready; ~10 short dummy
matmuls keep HAM at full clock through the initial DMA wait without
head-blocking the prefill; input DMAs are spread over the sync, scalar and
gpsimd rings so the scalar queue is clear before the exp stream starts;
row-3 out-projection is split so only one matmul + add + DMA per o-tile
remains after the final normalize, pipelined over psum/sbuf double-buffers
and both DMA rings.
"""

import numpy as np
import ml_dtypes

import concourse.bass as bass
import concourse.tile as tile
from concourse import bacc, mybir
from concourse.bass_utils import run_bass_kernel_spmd

P = 128          # partitions
S = 2048         # sequence length (one batch per core)
DIN = 1024       # model dim
DG = 512         # head-group width per core (8 heads x 64)
HD = 64          # head dim
NH = 8           # heads per core
QC = 512         # q-chunk (matmul free dim)
NQC = S // QC    # 4 q-chunks
NKT = S // P     # 16 k-tiles
KDT = DIN // P   # 8 din k-tiles
NHP = 4          # head pairs per core
F32 = mybir.dt.float32
BF16 = mybir.dt.bfloat16
FP8 = mybir.dt.float8e4
EXP = mybir.ActivationFunctionType.Exp
DR = mybir.MatmulPerfMode.DoubleRow

WSCALE = 64.0    # fp8 q/k weight pre-scale
N_WARM = 12      # dummy warm-up matmuls during the initial DMA wait; must
WARM_N = 512     # sustain >3.4us of PE activity to trip HAM to full clock
N_TAILWARM = 10  # dummy matmuls covering the last-normalize PE gap

_CACHE = {}


def _emit(tc, d):
    nc = tc.nc
    with (
        nc.allow_low_precision(reason="bf16 attention pipeline"),
        tc.tile_pool(name="persist", bufs=1) as pp,
        tc.tile_pool(name="work", bufs=4) as wp,
        tc.tile_pool(name="psc", bufs=2, space="PSUM") as psc,
        tc.tile_pool(name="ppj", bufs=2, space="PSUM") as ppj,
        tc.tile_pool(name="pcx", bufs=1, space="PSUM") as pcx,
    ):
        # ---- persistent SBUF tiles (layouts match the pre-tiled DRAM) ----
        xT = pp.tile([P, NQC, KDT, QC], BF16, tag="xT", name="xT")
        x8 = pp.tile([P, NQC, KDT, QC], FP8, tag="x8", name="x8")
        wq = pp.tile([P, NHP, KDT, P], FP8, tag="wq", name="wq")
        wk = pp.tile([P, NHP, KDT, P], FP8, tag="wk", name="wk")
        wv = pp.tile([P, KDT, DG], BF16, tag="wv", name="wv")
        wo = pp.tile([P, 4, DIN], BF16, tag="wo", name="wo")
        qT = [pp.tile([P, S], BF16, tag=f"qT{t}", name=f"qT{t}") for t in range(NHP)]
        kT = [pp.tile([P, S], BF16, tag=f"kT{t}", name=f"kT{t}") for t in range(NHP)]
        vv = [pp.tile([P, NH, HD + 1], BF16, tag=f"v{m}", name=f"v{m}") for m in range(NKT)]
        cx = [pp.tile([P, S], BF16, tag=f"cx{t}", name=f"cx{t}") for t in range(NHP)]
        ob3 = [pp.tile([P, QC], BF16, tag=f"ob3{o}", name=f"ob3{o}") for o in range(8)]
        msk = pp.tile([P, 2, P], BF16, tag="msk", name="msk")
        idt = pp.tile([P, P], BF16, tag="idt", name="idt")
        wrm = pp.tile([P, WARM_N], BF16, tag="wrm", name="wrm")

        # ---- PE warm-up: short garbage matmuls keep the PE busy (HAM at
        # full clock) while the first input DMAs land, without committing
        # the PE FIFO much past the x8 arrival ----
        nc.vector.memset(wrm[:], 0.0)
        for g in range(0, N_WARM, 5):
            ps = ppj.tile([P, WARM_N], F32, tag="pj", name="ps")
            n = min(5, N_WARM - g)
            for i in range(n):
                nc.tensor.matmul(
                    ps[:], wrm[:, 0:P], wrm[:],
                    start=(i == 0), stop=(i == n - 1),
                )

        # ---- input DMAs: big contiguous-run transfers, ordered by need,
        # critical prefill set first, spread across three HW-DGE rings so
        # the scalar queue is idle before the exp stream starts ----
        nc.sync.dma_start(x8[:, 0, 0:4, :], d["xq"][:, 0, 0:4, :])
        nc.scalar.dma_start(wq[:, 0, :, :], d["wqT"][:, 0, :, :])
        nc.scalar.dma_start(wk[:, 0, :, :], d["wkT"][:, 0, :, :])
        nc.sync.dma_start(x8[:, 0, 4:KDT, :], d["xq"][:, 0, 4:KDT, :])
        nc.scalar.dma_start(msk[:], d["masks"][:])
        nc.sync.dma_start(wv[:], d["wvT"][:])
        nc.sync.dma_start(xT[:, 0, :, :], d["xT"][:, 0, :, :])
        nc.scalar.dma_start(wq[:, 1:NHP, :, :], d["wqT"][:, 1:NHP, :, :])
        nc.scalar.dma_start(wk[:, 1:NHP, :, :], d["wkT"][:, 1:NHP, :, :])
        nc.scalar.dma_start(idt[:], d["ident"][:])
        for s in range(1, NQC):
            nc.sync.dma_start(xT[:, s, :, :], d["xT"][:, s, :, :])
            nc.gpsimd.dma_start(x8[:, s, :, :], d["xq"][:, s, :, :])
        nc.gpsimd.dma_start(wo[:], d["woT"][:])

        # ---- filler units ----
        def u_v(m):
            def f():
                ps = ppj.tile([P, QC], F32, tag="pj", name="ps")
                for k in range(KDT):
                    nc.tensor.matmul(
                        ps[:],
                        xT[:, m // 4, k, (m % 4) * P:(m % 4 + 1) * P],
                        wv[:, k, :],
                        start=(k == 0),
                        stop=(k == KDT - 1),
                    )
                nc.vector.tensor_copy(
                    vv[m][:, :, 0:HD], ps[:].rearrange("p (h e) -> p h e", h=NH)
                )
                nc.vector.memset(vv[m][:, :, HD:HD + 1], 1.0)
            return f

        def u_chain(t, w, s):
            def f():
                wt, dst = ((wq, qT), (wk, kT))[w]
                ps = ppj.tile([P, QC], F32, tag="pj", name="ps")
                for k in range(0, KDT, 2):
                    nc.tensor.matmul(
                        ps[:],
                        wt[:, t, k:k + 2, :],
                        x8[:, s, k:k + 2, :],
                        start=(k == 0),
                        stop=(k == KDT - 2),
                        perf_mode=DR,
                    )
                nc.vector.tensor_copy(dst[t][:, s * QC:(s + 1) * QC], ps[:])
            return f

        def u_out(s, o):
            def f():
                ps = ppj.tile([P, QC], F32, tag="pj", name="ps")
                for k in range(4):
                    nc.tensor.matmul(
                        ps[:],
                        wo[:, k, o * P:(o + 1) * P],
                        cx[k][:, s * QC:(s + 1) * QC],
                        start=(k == 0), stop=(k == 3),
                    )
                ob = wp.tile([P, QC], BF16, tag="ob", name="ob", bufs=2)
                nc.vector.tensor_copy(ob[:], ps[:])
                nc.sync.dma_start(
                    d["outT"][o * P:(o + 1) * P, s * QC:(s + 1) * QC], ob[:]
                )
            return f

        def u_out3_partial(o):
            # row-3 out-proj, head-pair groups 0..2 only -> SBUF partial
            def f():
                ps = ppj.tile([P, QC], F32, tag="pj", name="ps")
                for k in range(3):
                    nc.tensor.matmul(
                        ps[:],
                        wo[:, k, o * P:(o + 1) * P],
                        cx[k][:, 3 * QC:S],
                        start=(k == 0), stop=(k == 2),
                    )
                nc.vector.tensor_copy(ob3[o][:], ps[:])
            return f

        def u_out3_final(o):
            # row-3 matmul + the row-0..2 partial folded in via an identity
            # matmul (PE is idle at the tail, DVE is not); psum->SBUF copies
            # alternate DVE/ScalarE so the copy chains run in parallel
            ps = ppj.tile([P, QC], F32, tag="pj", name="ps")
            nc.tensor.matmul(
                ps[:], wo[:, 3, o * P:(o + 1) * P], cx[3][:, 3 * QC:S],
                start=True, stop=False,
            )
            nc.tensor.matmul(ps[:], idt[:], ob3[o][:], start=False, stop=True)
            ob = wp.tile([P, QC], BF16, tag="obf", name="obf", bufs=4)
            if o % 2 == 0:
                nc.vector.tensor_copy(ob[:], ps[:])
            else:
                nc.scalar.activation(
                    ob[:], ps[:], mybir.ActivationFunctionType.Copy
                )
            eng = nc.sync if o % 2 == 0 else nc.gpsimd
            eng.dma_start(d["outT"][o * P:(o + 1) * P, 3 * QC:S], ob[:])

        # consume-ordered filler queue + hard prerequisites per chunk
        queue = [u_v(0), u_v(1), u_v(2), u_v(3)]
        pre = {}
        for s in range(NQC):
            for hp in range(NHP):
                if (hp, s) == (0, 0):
                    pre[(hp, s)] = 0
                    continue
                if hp == 0 and s >= 1:
                    queue += [u_v(m) for m in range(4 * s, 4 * s + 4)]
                queue += [u_chain(hp, 0, s), u_chain(hp, 1, s)]
                pre[(hp, s)] = len(queue)
            if s == 1 or s == 2:
                queue += [u_out(s - 1, o) for o in range(8)]
        queue += [u_out(2, o) for o in range(8)]
        queue += [u_out3_partial(o) for o in range(8)]
        n_units = len(queue)

        order = [(hp, s) for s in range(NQC) for hp in range(NHP)]
        nxt = {order[i]: order[i + 1] for i in range(len(order) - 1)}

        state = {"drained": 0}

        def drain_to(idx):
            while state["drained"] < idx:
                queue[state["drained"]]()
                state["drained"] += 1

        def normalize(hp, s):
            last = (hp, s) == (NHP - 1, NQC - 1)
            cps = state["cps"]
            if last:
                # low-latency tail: DVE 32x32 block-transpose lifts the
                # [1,1024] sumexp row (row 64 = col 0 of the 32-aligned psum
                # window [64:96]; rows 65:96 are memset filler) onto 32
                # partitions straight out of PSUM, reciprocal in place
                # (col 0 -> col 1), transpose back, then multiply straight
                # from PSUM - no SBUF staging on the critical tail
                t1 = wp.tile([32, 2, QC], F32, tag="t1", name="t1", bufs=1)
                t2 = wp.tile([32, 2, QC], F32, tag="t2", name="t2", bufs=1)
                nc.vector.memset(t2[:], 1.0)  # early, off the critical path
                nc.vector.transpose(t1[:], cps[64:96, :, :])
                tv1 = t1[:].rearrange("p h (b j) -> p h b j", j=32)
                tv2 = t2[:].rearrange("p h (b j) -> p h b j", j=32)
                nc.vector.reciprocal(tv2[:, :, :, 0:1], tv1[:, :, :, 0:1])
                rc = wp.tile([32, 2, QC], F32, tag="rc", name="rc", bufs=1)
                nc.vector.transpose(rc[:], t2[:])
                bs = wp.tile([HD, 2, QC], F32, tag="bs", name="bs", bufs=2)
                nc.gpsimd.partition_broadcast(bs[:], rc[0:1, :, :])
                cxs = wp.tile([HD, QC], BF16, tag="cxs", name="cxs", bufs=2)
                nc.vector.tensor_mul(cxs[:], cps[0:HD, 1, :], bs[:, 1, :])
                nc.sync.dma_start(cx[hp][HD:P, s * QC:(s + 1) * QC], cxs[:])
                nc.vector.tensor_mul(
                    cx[hp][0:HD, s * QC:(s + 1) * QC], cps[0:HD, 0, :], bs[:, 0, :]
                )
                return
            cb = wp.tile([HD + 1, 2, QC], F32, tag="cb", name="cb", bufs=2)
            nc.vector.tensor_copy(cb[:], cps[0:HD + 1, :, :])
            zt = wp.tile([P, 8], F32, tag="zt", name="zt", bufs=2)
            nc.sync.dma_start(zt[:], cb[HD:HD + 1, :, :])
            rt = wp.tile([P, 8], F32, tag="rt", name="rt", bufs=2)
            nc.vector.reciprocal(rt[:], zt[:])
            rr = wp.tile([1, 2, QC], F32, tag="rr", name="rr", bufs=2)
            nc.sync.dma_start(rr[:], rt[:])
            bs = wp.tile([HD, 2, QC], F32, tag="bs", name="bs", bufs=2)
            nc.gpsimd.partition_broadcast(bs[:], rr[:])
            # head B first: its partition-shift DMA overlaps head A's mul
            cxs = wp.tile([HD, QC], BF16, tag="cxs", name="cxs", bufs=2)
            nc.vector.tensor_mul(cxs[:], cb[0:HD, 1, :], bs[:, 1, :])
            nc.sync.dma_start(cx[hp][HD:P, s * QC:(s + 1) * QC], cxs[:])
            nc.vector.tensor_mul(
                cx[hp][0:HD, s * QC:(s + 1) * QC], cb[0:HD, 0, :], bs[:, 0, :]
            )

        def attn_chunk(hp, s):
            t0 = pre[(hp, s)]
            t1 = pre[nxt[(hp, s)]] if (hp, s) in nxt else n_units
            nkt = 4 * (s + 1)  # causal: k-tiles 0..nkt-1
            cps = pcx.tile([96, 2, QC], F32, tag="cx", name="cps")
            state["cps"] = cps
            if (hp, s) == (NHP - 1, NQC - 1):
                # valid filler above the sumexp row for the tail transpose
                # (32-aligned window; row 64 is re-written by the start=True
                # attn*V accumulation right after)
                nc.vector.memset(cps[HD:96, :, :], 1.0)

            def attn_v_pair(pair):
                for k, s0, a in pair:
                    nc.tensor.matmul(
                        cps[0:HD + 1, 0, s0:], vv[k][:, 2 * hp, :], a[:, 0, s0:],
                        start=(k == 0), stop=(k == nkt - 1),
                    )
                    nc.tensor.matmul(
                        cps[0:HD + 1, 1, s0:], vv[k][:, 2 * hp + 1, :], a[:, 1, s0:],
                        start=(k == 0), stop=(k == nkt - 1),
                    )

            pend = None  # attn*V emitted one k-tile-PAIR late: its exp and
            # mask are always done by the time it reaches the head of the
            # FIFO tensor queue, so it never head-blocks the scores stream
            for pk in range(nkt // 2):
                tiles = []
                # scores for both k-tiles of the pair back-to-back: keeps
                # the row-tiled LDWEIGHTS adjacent (one full-array<->row-
                # group transition per pair instead of two)
                for k in (2 * pk, 2 * pk + 1):
                    dd = k - 4 * s
                    s0 = max(dd, 0) * P  # causal q-range restriction
                    sps = psc.tile([P, 2, QC], F32, tag="sc", name="sps")
                    nc.tensor.matmul(
                        sps[:, 0, s0:],
                        kT[hp][0:HD, k * P:(k + 1) * P],
                        qT[hp][0:HD, s * QC + s0:(s + 1) * QC],
                        start=True, stop=True,
                    )
                    nc.tensor.matmul(
                        sps[:, 1, s0:],
                        kT[hp][HD:P, k * P:(k + 1) * P],
                        qT[hp][HD:P, s * QC + s0:(s + 1) * QC],
                        start=True, stop=True,
                    )
                    tiles.append((k, s0, sps))
                cur = []
                for k, s0, sps in tiles:
                    a = wp.tile([P, 2, QC], BF16, tag="a", name="a", bufs=5)
                    nc.scalar.activation(
                        a[:, :, s0:], sps[:, :, s0:], EXP, scale=d["escale"]
                    )
                    if k - 4 * s >= 0:
                        # only columns [s0, s0+128) straddle the diagonal;
                        # the block mask is the same triangle for every dd
                        nc.vector.tensor_mul(
                            a[:, :, s0:s0 + P], a[:, :, s0:s0 + P], msk[:]
                        )
                    cur.append((k, s0, a))
                if pend is not None:
                    attn_v_pair(pend)
                pend = cur
                drain_to(min(t1, t0 + ((t1 - t0) * (2 * pk + 4)) // nkt,
                             state["drained"] + 4))
            attn_v_pair(pend)
            normalize(hp, s)

        # ---- prefill: just enough to start chunk (0,0) ----
        u_chain(0, 0, 0)()
        u_chain(0, 1, 0)()

        # ---- main stream ----
        for s in range(NQC):
            for hp in range(NHP):
                drain_to(pre[(hp, s)])
                attn_chunk(hp, s)
        drain_to(n_units)
        # dummy matmuls bridge the PE gap while the final normalize runs so
        # HAM stays at full clock for the final out-projection row
        for g in range(0, N_TAILWARM, 5):
            ps = ppj.tile([P, QC], F32, tag="pj", name="dum")
            n = min(5, N_TAILWARM - g)
            for i in range(n):
                nc.tensor.matmul(
                    ps[:], wrm[:, 0:P], wrm[:],
                    start=(i == 0), stop=(i == n - 1),
                )
        for o in range(8):
            u_out3_final(o)


def _build():
    if "nc" in _CACHE:
        return _CACHE["nc"]
    nc = bacc.Bacc("TRN2", target_bir_lowering=False, debug=False, num_devices=8)
    d = {
        "xT": nc.dram_tensor("xT", [P, NQC, KDT, QC], BF16, kind="ExternalInput").ap(),
        "wvT": nc.dram_tensor("wvT", [P, KDT, DG], BF16, kind="ExternalInput").ap(),
        "woT": nc.dram_tensor("woT", [P, 4, DIN], BF16, kind="ExternalInput").ap(),
        "masks": nc.dram_tensor("masks", [P, 2, P], BF16, kind="ExternalInput").ap(),
        "ident": nc.dram_tensor("ident", [P, P], BF16, kind="ExternalInput").ap(),
        "outT": nc.dram_tensor("outT", [DIN, S], BF16, kind="ExternalOutput").ap(),
        "wqT": nc.dram_tensor("wqT", [P, NHP, KDT, P], FP8, kind="ExternalInput").ap(),
        "wkT": nc.dram_tensor("wkT", [P, NHP, KDT, P], FP8, kind="ExternalInput").ap(),
        "xq": nc.dram_tensor("xq", [P, NQC, KDT, QC], FP8, kind="ExternalInput").ap(),
        "escale": 0.125 / (WSCALE * WSCALE),
    }
    with tile.TileContext(nc) as tc:
        _emit(tc, d)
    nc.compile()
    _CACHE["nc"] = nc
    return nc


def _masks_np():
    r = np.arange(P)[:, None]
    j = np.arange(P)[None, :]
    m = (j >= r).astype(ml_dtypes.bfloat16)  # [128, 128] upper triangle
    return np.ascontiguousarray(np.broadcast_to(m[:, None, :], (P, 2, P)))


def _tile_k(a, kdt=KDT):
    """[kdt*P, C] -> [P, kdt, C] (din-subtile blocking)."""
    c = a.shape[1]
    return np.ascontiguousarray(a.reshape(kdt, P, c).transpose(1, 0, 2))


def _f8(a):
    return np.clip(a, -240, 240).astype(ml_dtypes.float8_e4m3)


def kernel(x, Wq, Wk, Wv, Wo, bo, _run_kwargs=None, _return_res=False):
    x = np.asarray(x)
    Wq, Wk, Wv, Wo, bo = (np.asarray(a) for a in (Wq, Wk, Wv, Wo, bo))
    B = x.shape[0]
    nc = _build()

    def b16(a):
        return np.ascontiguousarray(a).astype(ml_dtypes.bfloat16)

    masks = _masks_np()
    in_maps = []
    for c in range(8):
        b, g = divmod(c, 2)
        xt = b16(x[b].T)  # [1024, 2048]
        xt4 = xt.reshape(KDT, P, NQC, QC).transpose(1, 2, 0, 3)  # [p,s,k,c]
        wqt = Wq[g * DG:(g + 1) * DG, :].T  # [1024, 512] f32
        wkt = Wk[g * DG:(g + 1) * DG, :].T
        im = {
            "xT": np.ascontiguousarray(xt4),
            "wvT": _tile_k(b16(Wv[g * DG:(g + 1) * DG, :].T)),
            "woT": _tile_k(b16(Wo[:, g * DG:(g + 1) * DG].T), kdt=4),
            "masks": masks,
            "ident": np.eye(P, dtype=ml_dtypes.bfloat16),
            "xq": np.ascontiguousarray(
                _f8(x[b].T).reshape(KDT, P, NQC, QC).transpose(1, 2, 0, 3)),
            "wqT": np.ascontiguousarray(
                _f8(WSCALE * wqt).reshape(KDT, P, NHP, P).transpose(1, 2, 0, 3)),
            "wkT": np.ascontiguousarray(
                _f8(WSCALE * wkt).reshape(KDT, P, NHP, P).transpose(1, 2, 0, 3)),
        }
        in_maps.append(im)

    res = run_bass_kernel_spmd(nc, in_maps, list(range(8)), **(_run_kwargs or {}))
    out = np.empty((B, S, DIN), np.float32)
    for b in range(B):
        p = (res.results[2 * b]["outT"].astype(np.float32)
             + res.results[2 * b + 1]["outT"].astype(np.float32))
        out[b] = p.T + bo.astype(np.float32)
    if _return_res:
        return out, res
    return out


# revision 26
# speedup vs baseline: 1.0796x; 1.0094x over previous
"""Causal multi-head attention (B=4, S=2048, D=1024, H=16, hd=64) on 8 TRN2
NeuronCores.

Sharding: core c = (batch b = c//2, head-group g = c%2). Each core computes
QKV projections for its 8 heads (Megatron column-split), causal attention,
and a partial out-projection (row-split); the host sums the two head-group
partials per batch and adds the bias.

On-device layout (bf16 compute, fp32 PSUM accumulation):
  xT  [p, q-block, din-subtile, 512]  x[b]^T pre-tiled on host so each
        input DMA moves 8KB-contiguous runs per partition (descriptor-
        efficient); same for wq/wk (head-pair-blocked), wv, wo
  q/k projections in fp8-e4m3 DoubleRow (weights pre-scaled x64 on host,
        the exp scale absorbs the 1/4096; value path stays bf16 - fp8
        anywhere in v/attn costs ~1% extra rel err, over budget)
  qT/kT as [d_g, S] transposed tiles: head-pair t -> partitions
        [0:64] head 2t, [64:128] head 2t+1
  v   [k-tile 128, 8 heads, 65]: col 64 is ones (sumexp lands in the ctx^T
        psum row 64 for free during the attn*V matmul)
  scores^T psum tiles [k 128, 2 heads, q 512]: head pair packed via PE row
        tiling (K=64 each, concurrent).  Scores for TWO k-tiles are emitted
        back-to-back: full-array<->row-group LDWEIGHTS transitions stall
        ~100ns each (the PE can only pull an LDW ahead of in-flight matmuls
        into a non-conflicting row group), so batching the row-tiled pairs
        halves the number of transitions.
  attn = exp(scores/8) per k-tile on ScalarE; causal via skipping k-tiles
        above the diagonal, restricting the q-range on diagonal tiles, and
        one masked multiply per diagonal tile (the 128-wide diagonal block
        is the same upper triangle for every dd, both heads in one op)
  ctx^T accumulated in PSUM over k-tiles; normalize via DRAM-roundtrip
        reciprocal + gpsimd partition broadcast.  The last chunk instead
        transposes the sumexp row straight out of PSUM (DVE 32x32 block
        transpose), reciprocates in place, transposes back, and multiplies
        straight from PSUM - no SBUF copies on the critical tail.

Schedule: all non-attention matmuls drain as filler INSIDE the attention
stream (interpolated between per-chunk prerequisite markers); attn*V is
emitted one k-tile-PAIR late so its exp is always ready; ~10 short dummy
matmuls keep HAM at full clock through the initial DMA wait without
head-blocking the prefill; input DMAs are spread over the sync, scalar and
gpsimd rings so the scalar queue is clear before the exp stream starts;
row-3 out-projection is split so only one matmul + add + DMA per o-tile
remains after the final normalize, pipelined over psum/sbuf double-buffers
and both DMA rings.
"""

import numpy as np
import ml_dtypes

import concourse.bass as bass
import concourse.tile as tile
from concourse import bacc, mybir
from concourse.bass_utils import run_bass_kernel_spmd

P = 128          # partitions
S = 2048         # sequence length (one batch per core)
DIN = 1024       # model dim
DG = 512         # head-group width per core (8 heads x 64)
HD = 64          # head dim
NH = 8           # heads per core
QC = 512         # q-chunk (matmul free dim)
NQC = S // QC    # 4 q-chunks
NKT = S // P     # 16 k-tiles
KDT = DIN // P   # 8 din k-tiles
NHP = 4          # head pairs per core
F32 = mybir.dt.float32
BF16 = mybir.dt.bfloat16
FP8 = mybir.dt.float8e4
EXP = mybir.ActivationFunctionType.Exp
DR = mybir.MatmulPerfMode.DoubleRow

WSCALE = 64.0    # fp8 q/k weight pre-scale
N_WARM = 8       # dummy warm-up matmuls during the initial DMA wait; must
WARM_N = 512     # sustain >3.4us of PE activity to trip HAM to full clock
N_TAILWARM = 18  # dummy matmuls covering the last-normalize PE gap

_CACHE = {}


def _emit(tc, d):
    nc = tc.nc
    with (
        nc.allow_low_precision(reason="bf16 attention pipeline"),
        tc.tile_pool(name="persist", bufs=1) as pp,
        tc.tile_pool(name="work", bufs=4) as wp,
        tc.tile_pool(name="psc", bufs=2, space="PSUM") as psc,
        tc.tile_pool(name="ppj", bufs=2, space="PSUM") as ppj,
        tc.tile_pool(name="pcx", bufs=1, space="PSUM") as pcx,
    ):
        # ---- persistent SBUF tiles (layouts match the pre-tiled DRAM) ----
        xT = pp.tile([P, NQC, KDT, QC], BF16, tag="xT", name="xT")
        x8 = pp.tile([P, NQC, KDT, QC], FP8, tag="x8", name="x8")
        wq = pp.tile([P, NHP, KDT, P], FP8, tag="wq", name="wq")
        wk = pp.tile([P, NHP, KDT, P], FP8, tag="wk", name="wk")
        wv = pp.tile([P, KDT, DG], BF16, tag="wv", name="wv")
        wo = pp.tile([P, 4, DIN], BF16, tag="wo", name="wo")
        qT = [pp.tile([P, S], BF16, tag=f"qT{t}", name=f"qT{t}") for t in range(NHP)]
        kT = [pp.tile([P, S], BF16, tag=f"kT{t}", name=f"kT{t}") for t in range(NHP)]
        vv = [pp.tile([P, NH, HD + 1], BF16, tag=f"v{m}", name=f"v{m}") for m in range(NKT)]
        cx = [pp.tile([P, S], BF16, tag=f"cx{t}", name=f"cx{t}") for t in range(NHP)]
        ob3 = pp.tile([P, 8, QC], BF16, tag="ob3", name="ob3")
        msk = pp.tile([P, 2, P], BF16, tag="msk", name="msk")
        idt = pp.tile([P, P], BF16, tag="idt", name="idt")
        wrm = pp.tile([P, WARM_N], BF16, tag="wrm", name="wrm")

        # ---- PE warm-up: garbage matmuls (the psum is never read) keep the
        # PE busy from the earliest possible moment so HAM reaches full
        # clock before the first chains, without committing the PE FIFO
        # much past the x8 arrival ----
        nc.vector.memset(wrm[:], 0.0)
        for g in range(0, N_WARM, 5):
            ps = ppj.tile([P, WARM_N], F32, tag="pj", name="ps")
            n = min(5, N_WARM - g)
            for i in range(n):
                nc.tensor.matmul(
                    ps[:], wrm[:, 0:P], wrm[:],
                    start=(i == 0), stop=(i == n - 1),
                )

        # ---- input DMAs: big contiguous-run transfers, ordered by need,
        # critical prefill set first, spread across three HW-DGE rings so
        # the scalar queue is idle before the exp stream starts ----
        nc.sync.dma_start(x8[:, 0, 0:4, :], d["xq"][:, 0, 0:4, :])
        nc.scalar.dma_start(wq[:, 0, :, :], d["wqT"][:, 0, :, :])
        nc.scalar.dma_start(wk[:, 0, :, :], d["wkT"][:, 0, :, :])
        nc.sync.dma_start(x8[:, 0, 4:KDT, :], d["xq"][:, 0, 4:KDT, :])
        nc.scalar.dma_start(msk[:], d["masks"][:])
        nc.sync.dma_start(wv[:], d["wvT"][:])
        nc.sync.dma_start(xT[:, 0, :, :], d["xT"][:, 0, :, :])
        nc.scalar.dma_start(wq[:, 1:NHP, :, :], d["wqT"][:, 1:NHP, :, :])
        nc.scalar.dma_start(wk[:, 1:NHP, :, :], d["wkT"][:, 1:NHP, :, :])
        nc.scalar.dma_start(idt[:], d["ident"][:])
        # deferred inputs ride the scalar ring BEHIND the critical weights:
        # the serial ring is a priority queue, while a separate (empty) ring
        # would fire immediately and steal HBM bandwidth from the critical
        # x8/xT stream on the sync ring
        for s in range(1, NQC):
            nc.sync.dma_start(xT[:, s, :, :], d["xT"][:, s, :, :])
            nc.scalar.dma_start(x8[:, s, :, :], d["xq"][:, s, :, :])
        nc.scalar.dma_start(wo[:], d["woT"][:])

        # ---- filler units ----
        def u_v(m):
            def f():
                ps = ppj.tile([P, QC], F32, tag="pj", name="ps")
                for k in range(KDT):
                    nc.tensor.matmul(
                        ps[:],
                        xT[:, m // 4, k, (m % 4) * P:(m % 4 + 1) * P],
                        wv[:, k, :],
                        start=(k == 0),
                        stop=(k == KDT - 1),
                    )
                nc.vector.tensor_copy(
                    vv[m][:, :, 0:HD], ps[:].rearrange("p (h e) -> p h e", h=NH)
                )
                nc.vector.memset(vv[m][:, :, HD:HD + 1], 1.0)
            return f

        def u_chain(t, w, s):
            def f():
                wt, dst = ((wq, qT), (wk, kT))[w]
                ps = ppj.tile([P, QC], F32, tag="pj", name="ps")
                for k in range(0, KDT, 2):
                    nc.tensor.matmul(
                        ps[:],
                        wt[:, t, k:k + 2, :],
                        x8[:, s, k:k + 2, :],
                        start=(k == 0),
                        stop=(k == KDT - 2),
                        perf_mode=DR,
                    )
                nc.vector.tensor_copy(dst[t][:, s * QC:(s + 1) * QC], ps[:])
            return f

        def u_out(s, o):
            def f():
                ps = ppj.tile([P, QC], F32, tag="pj", name="ps")
                for k in range(4):
                    nc.tensor.matmul(
                        ps[:],
                        wo[:, k, o * P:(o + 1) * P],
                        cx[k][:, s * QC:(s + 1) * QC],
                        start=(k == 0), stop=(k == 3),
                    )
                ob = wp.tile([P, QC], BF16, tag="ob", name="ob", bufs=2)
                nc.vector.tensor_copy(ob[:], ps[:])
                nc.sync.dma_start(
                    d["outT"][o * P:(o + 1) * P, s * QC:(s + 1) * QC], ob[:]
                )
            return f

        def u_out3_partial(o):
            # row-3 out-proj, head-pair groups 0..2 only -> SBUF partial
            def f():
                ps = ppj.tile([P, QC], F32, tag="pj", name="ps")
                for k in range(3):
                    nc.tensor.matmul(
                        ps[:],
                        wo[:, k, o * P:(o + 1) * P],
                        cx[k][:, 3 * QC:S],
                        start=(k == 0), stop=(k == 2),
                    )
                nc.vector.tensor_copy(ob3[:, o, :], ps[:])
            return f

        def u_out3_final(t):
            # o-tile pair 2t/2t+1: two row-3 matmuls + ONE identity matmul
            # folding in the row-0..2 partials (PE is idle at the tail, DVE
            # is not); psum->SBUF copies alternate DVE/ScalarE so the two
            # copy chains run in parallel
            ps = psc.tile([P, 2, QC], F32, tag="sc", name="fin")
            for j in range(2):
                o = 2 * t + j
                nc.tensor.matmul(
                    ps[:, j, :], wo[:, 3, o * P:(o + 1) * P], cx[3][:, 3 * QC:S],
                    start=True, stop=False,
                )
                nc.tensor.matmul(
                    ps[:, j, :], idt[:], ob3[:, o, :], start=False, stop=True,
                )
            ob = wp.tile([P, 2, QC], BF16, tag="obf", name="obf", bufs=2)
            if t % 2 == 0:
                nc.vector.tensor_copy(ob[:], ps[:])
            else:
                nc.scalar.activation(
                    ob[:], ps[:], mybir.ActivationFunctionType.Copy
                )
            eng = nc.sync if t % 2 == 0 else nc.gpsimd
            dst = d["outT"][2 * t * P:(2 * t + 2) * P, 3 * QC:S]
            eng.dma_start(dst.rearrange("(o p) q -> p o q", o=2), ob[:])

        # consume-ordered filler queue + hard prerequisites per chunk
        queue = [u_v(0), u_v(1), u_v(2), u_v(3)]
        pre = {}
        for s in range(NQC):
            for hp in range(NHP):
                if (hp, s) == (0, 0):
                    pre[(hp, s)] = 0
                    continue
                if hp == 0 and s >= 1:
                    queue += [u_v(m) for m in range(4 * s, 4 * s + 4)]
                queue += [u_chain(hp, 0, s), u_chain(hp, 1, s)]
                pre[(hp, s)] = len(queue)
            if s == 1 or s == 2:
                queue += [u_out(s - 1, o) for o in range(8)]
        queue += [u_out(2, o) for o in range(8)]
        queue += [u_out3_partial(o) for o in range(8)]
        n_units = len(queue)

        order = [(hp, s) for s in range(NQC) for hp in range(NHP)]
        nxt = {order[i]: order[i + 1] for i in range(len(order) - 1)}

        state = {"drained": 0}

        def drain_to(idx):
            while state["drained"] < idx:
                queue[state["drained"]]()
                state["drained"] += 1

        def normalize(hp, s):
            last = (hp, s) == (NHP - 1, NQC - 1)
            cps = state["cps"]
            if last:
                # low-latency tail: DVE 32x32 block-transpose lifts the
                # [1,1024] sumexp row (row 64 = col 0 of the 32-aligned psum
                # window [64:96]; rows 65:96 are memset filler) onto 32
                # partitions straight out of PSUM, reciprocal in place
                # (col 0 -> col 1), transpose back, then multiply straight
                # from PSUM - no SBUF staging on the critical tail
                t1 = wp.tile([32, 2, QC], F32, tag="t1", name="t1", bufs=1)
                t2 = wp.tile([32, 2, QC], F32, tag="t2", name="t2", bufs=1)
                nc.vector.memset(t2[:], 1.0)  # early, off the critical path
                nc.vector.transpose(t1[:], cps[64:96, :, :])
                tv1 = t1[:].rearrange("p h (b j) -> p h b j", j=32)
                tv2 = t2[:].rearrange("p h (b j) -> p h b j", j=32)
                nc.vector.reciprocal(tv2[:, :, :, 0:1], tv1[:, :, :, 0:1])
                rc = wp.tile([32, 2, QC], F32, tag="rc", name="rc", bufs=1)
                nc.vector.transpose(rc[:], t2[:])
                bs = wp.tile([HD, 2, QC], F32, tag="bs", name="bs", bufs=2)
                nc.gpsimd.partition_broadcast(bs[:], rc[0:1, :, :])
                cxs = wp.tile([HD, QC], BF16, tag="cxs", name="cxs", bufs=2)
                nc.vector.tensor_mul(cxs[:], cps[0:HD, 1, :], bs[:, 1, :])
                nc.sync.dma_start(cx[hp][HD:P, s * QC:(s + 1) * QC], cxs[:])
                nc.vector.tensor_mul(
                    cx[hp][0:HD, s * QC:(s + 1) * QC], cps[0:HD, 0, :], bs[:, 0, :]
                )
                return
            cb = wp.tile([HD + 1, 2, QC], F32, tag="cb", name="cb", bufs=2)
            nc.vector.tensor_copy(cb[:], cps[0:HD + 1, :, :])
            zt = wp.tile([P, 8], F32, tag="zt", name="zt", bufs=2)
            nc.sync.dma_start(zt[:], cb[HD:HD + 1, :, :])
            rt = wp.tile([P, 8], F32, tag="rt", name="rt", bufs=2)
            nc.vector.reciprocal(rt[:], zt[:])
            rr = wp.tile([1, 2, QC], F32, tag="rr", name="rr", bufs=2)
            nc.sync.dma_start(rr[:], rt[:])
            bs = wp.tile([HD, 2, QC], F32, tag="bs", name="bs", bufs=2)
            nc.gpsimd.partition_broadcast(bs[:], rr[:])
            # head B first: its partition-shift DMA overlaps head A's mul
            cxs = wp.tile([HD, QC], BF16, tag="cxs", name="cxs", bufs=2)
            nc.vector.tensor_mul(cxs[:], cb[0:HD, 1, :], bs[:, 1, :])
            nc.sync.dma_start(cx[hp][HD:P, s * QC:(s + 1) * QC], cxs[:])
            nc.vector.tensor_mul(
                cx[hp][0:HD, s * QC:(s + 1) * QC], cb[0:HD, 0, :], bs[:, 0, :]
            )

        def attn_chunk(hp, s):
            t0 = pre[(hp, s)]
            t1 = pre[nxt[(hp, s)]] if (hp, s) in nxt else n_units
            nkt = 4 * (s + 1)  # causal: k-tiles 0..nkt-1
            cps = pcx.tile([96, 2, QC], F32, tag="cx", name="cps")
            state["cps"] = cps
            if (hp, s) == (NHP - 1, NQC - 1):
                # valid filler above the sumexp row for the tail transpose
                # (32-aligned window; row 64 is re-written by the start=True
                # attn*V accumulation right after)
                nc.vector.memset(cps[HD:96, :, :], 1.0)

            def attn_v_pair(pair):
                for k, s0, a in pair:
                    nc.tensor.matmul(
                        cps[0:HD + 1, 0, s0:], vv[k][:, 2 * hp, :], a[:, 0, s0:],
                        start=(k == 0), stop=(k == nkt - 1),
                    )
                    nc.tensor.matmul(
                        cps[0:HD + 1, 1, s0:], vv[k][:, 2 * hp + 1, :], a[:, 1, s0:],
                        start=(k == 0), stop=(k == nkt - 1),
                    )

            pend = None  # attn*V emitted one k-tile-PAIR late: its exp and
            # mask are always done by the time it reaches the head of the
            # FIFO tensor queue, so it never head-blocks the scores stream
            for pk in range(nkt // 2):
                tiles = []
                # scores for both k-tiles of the pair back-to-back: keeps
                # the row-tiled LDWEIGHTS adjacent (one full-array<->row-
                # group transition per pair instead of two)
                for k in (2 * pk, 2 * pk + 1):
                    dd = k - 4 * s
                    s0 = max(dd, 0) * P  # causal q-range restriction
                    sps = psc.tile([P, 2, QC], F32, tag="sc", name="sps")
                    nc.tensor.matmul(
                        sps[:, 0, s0:],
                        kT[hp][0:HD, k * P:(k + 1) * P],
                        qT[hp][0:HD, s * QC + s0:(s + 1) * QC],
                        start=True, stop=True,
                    )
                    nc.tensor.matmul(
                        sps[:, 1, s0:],
                        kT[hp][HD:P, k * P:(k + 1) * P],
                        qT[hp][HD:P, s * QC + s0:(s + 1) * QC],
                        start=True, stop=True,
                    )
                    tiles.append((k, s0, sps))
                cur = []
                for k, s0, sps in tiles:
                    a = wp.tile([P, 2, QC], BF16, tag="a", name="a", bufs=5)
                    nc.scalar.activation(
                        a[:, :, s0:], sps[:, :, s0:], EXP, scale=d["escale"]
                    )
                    if k - 4 * s >= 0:
                        # only columns [s0, s0+128) straddle the diagonal;
                        # the block mask is the same triangle for every dd
                        nc.vector.tensor_mul(
                            a[:, :, s0:s0 + P], a[:, :, s0:s0 + P], msk[:]
                        )
                    cur.append((k, s0, a))
                if pend is not None:
                    attn_v_pair(pend)
                pend = cur
                drain_to(min(t1, t0 + ((t1 - t0) * (2 * pk + 4)) // nkt,
                             state["drained"] + 4))
            attn_v_pair(pend)
            normalize(hp, s)

        # ---- prefill: just enough to start chunk (0,0) ----
        u_chain(0, 0, 0)()
        u_chain(0, 1, 0)()

        # ---- main stream ----
        for s in range(NQC):
            for hp in range(NHP):
                drain_to(pre[(hp, s)])
                attn_chunk(hp, s)
        drain_to(n_units)
        # dummy matmuls bridge the PE gap while the final normalize runs so
        # HAM stays at full clock for the final out-projection row
        for g in range(0, N_TAILWARM, 5):
            ps = ppj.tile([P, QC], F32, tag="pj", name="dum")
            n = min(5, N_TAILWARM - g)
            for i in range(n):
                nc.tensor.matmul(
                    ps[:], wrm[:, 0:P], wrm[:],
                    start=(i == 0), stop=(i == n - 1),
                )
        for t in range(4):
            u_out3_final(t)


def _build():
    if "nc" in _CACHE:
        return _CACHE["nc"]
    nc = bacc.Bacc("TRN2", target_bir_lowering=False, debug=False, num_devices=8)
    d = {
        "xT": nc.dram_tensor("xT", [P, NQC, KDT, QC], BF16, kind="ExternalInput").ap(),
        "wvT": nc.dram_tensor("wvT", [P, KDT, DG], BF16, kind="ExternalInput").ap(),
        "woT": nc.dram_tensor("woT", [P, 4, DIN], BF16, kind="ExternalInput").ap(),
        "masks": nc.dram_tensor("masks", [P, 2, P], BF16, kind="ExternalInput").ap(),
        "ident": nc.dram_tensor("ident", [P, P], BF16, kind="ExternalInput").ap(),
        "outT": nc.dram_tensor("outT", [DIN, S], BF16, kind="ExternalOutput").ap(),
        "wqT": nc.dram_tensor("wqT", [P, NHP, KDT, P], FP8, kind="ExternalInput").ap(),
        "wkT": nc.dram_tensor("wkT", [P, NHP, KDT, P], FP8, kind="ExternalInput").ap(),
        "xq": nc.dram_tensor("xq", [P, NQC, KDT, QC], FP8, kind="ExternalInput").ap(),
        "escale": 0.125 / (WSCALE * WSCALE),
    }
    with tile.TileContext(nc) as tc:
        _emit(tc, d)
    nc.compile()
    _CACHE["nc"] = nc
    return nc


def _masks_np():
    r = np.arange(P)[:, None]
    j = np.arange(P)[None, :]
    m = (j >= r).astype(ml_dtypes.bfloat16)  # [128, 128] upper triangle
    return np.ascontiguousarray(np.broadcast_to(m[:, None, :], (P, 2, P)))


def _tile_k(a, kdt=KDT):
    """[kdt*P, C] -> [P, kdt, C] (din-subtile blocking)."""
    c = a.shape[1]
    return np.ascontiguousarray(a.reshape(kdt, P, c).transpose(1, 0, 2))


def _f8(a):
    return np.clip(a, -240, 240).astype(ml_dtypes.float8_e4m3)


def kernel(x, Wq, Wk, Wv, Wo, bo, _run_kwargs=None, _return_res=False):
    x = np.asarray(x)
    Wq, Wk, Wv, Wo, bo = (np.asarray(a) for a in (Wq, Wk, Wv, Wo, bo))
    B = x.shape[0]
    nc = _build()

    def b16(a):
        return np.ascontiguousarray(a).astype(ml_dtypes.bfloat16)

    masks = _masks_np()
    in_maps = []
    for c in range(8):
        b, g = divmod(c, 2)
        xt = b16(x[b].T)  # [1024, 2048]
        xt4 = xt.reshape(KDT, P, NQC, QC).transpose(1, 2, 0, 3)  # [p,s,k,c]
        wqt = Wq[g * DG:(g + 1) * DG, :].T  # [1024, 512] f32
        wkt = Wk[g * DG:(g + 1) * DG, :].T
        im = {
            "xT": np.ascontiguousarray(xt4),
            "wvT": _tile_k(b16(Wv[g * DG:(g + 1) * DG, :].T)),
            "woT": _tile_k(b16(Wo[:, g * DG:(g + 1) * DG].T), kdt=4),
            "masks": masks,
            "ident": np.eye(P, dtype=ml_dtypes.bfloat16),
            "xq": np.ascontiguousarray(
                _f8(x[b].T).reshape(KDT, P, NQC, QC).transpose(1, 2, 0, 3)),
            "wqT": np.ascontiguousarray(
                _f8(WSCALE * wqt).reshape(KDT, P, NHP, P).transpose(1, 2, 0, 3)),
            "wkT": np.ascontiguousarray(
                _f8(WSCALE * wkt).reshape(KDT, P, NHP, P).transpose(1, 2, 0, 3)),
        }
        in_maps.append(im)

    res = run_bass_kernel_spmd(nc, in_maps, list(range(8)), **(_run_kwargs or {}))
    out = np.empty((B, S, DIN), np.float32)
    for b in range(B):
        p = (res.results[2 * b]["outT"].astype(np.float32)
             + res.results[2 * b + 1]["outT"].astype(np.float32))
        out[b] = p.T + bo.astype(np.float32)
    if _return_res:
        return out, res
    return out
